# revision 1
# baseline (speedup 1.0000x reference)
"""Trainium2 Bass kernel for per-node multi-head attention.

Computation (per node n, fully independent across nodes):
    Q = h @ Wq.T  viewed (nh, hd)        [row-major reshape]
    K = h @ Wk.T  viewed (hd, nh)
    V = h @ Wv.T  viewed (hd, nh)
    comp[hh, g] = sum_d Q[hh, d] K[d, g] / 128
    scores = softmax(comp, axis=-1)
    out[l, d]  = sum_g scores[l, g] V[d, g]
    final = flat(out.T) @ Wfc.T

Sharding: data-parallel over the node dim N across 8 NeuronCores; weights
replicated; no collectives.

Per-core mapping:
  - TensorE: transpose h blocks (128x128), the 3 projections + final FC as
    float32r matmuls (full-rate fp32), transpose of the attention output.
  - VectorE: batched per-node einsums as broadcast-AP elementwise products
    (bf16) + segmented tensor_reduce, plus softmax normalization.
  - ScalarE: exp, PSUM->SBUF copies (with dtype casts).
  - Wk's rows are permuted during on-device weight prep so the K projection
    lands g-major (f' = g*64 + d), which makes the comp product APs unit-
    stride in the innermost dim.
"""

import numpy as np

N_FULL = 65536
H = 1024
NCORES = 8
NPC = N_FULL // NCORES  # rows per core
NH = 16                 # heads
HD = 64                 # head dim
KT = H // 128           # k tiles per contraction (8)

_BUILD_CACHE = {}


def _build(n_rows, ablate=()):
    """ablate: subset of {'attn', 'mm', 'tpose', 'copies'} — skip those parts
    (wrong results; used only for TimelineSim attribution experiments)."""
    key = (n_rows, tuple(sorted(ablate)))
    if key in _BUILD_CACHE:
        return _BUILD_CACHE[key]

    import concourse.bass as bass
    import concourse.mybir as mybir
    import concourse.tile as tile
    from concourse import bacc
    from concourse.masks import make_identity

    f32 = mybir.dt.float32
    f32r = mybir.dt.float32r
    bf16 = mybir.dt.bfloat16
    MULT = mybir.AluOpType.mult
    ADD = mybir.AluOpType.add
    AXX = mybir.AxisListType.X

    nc = bacc.Bacc("TRN2", target_bir_lowering=False, debug=False)

    h_d = nc.dram_tensor("h", [n_rows, H], f32, kind="ExternalInput").ap()
    w_d = {
        name: nc.dram_tensor(name, [H, H], f32, kind="ExternalInput").ap()
        for name in ("wq", "wk", "wv", "wfc")
    }
    out_d = nc.dram_tensor("out", [n_rows, H], f32, kind="ExternalOutput").ap()

    ntiles = n_rows // 128

    def ap(base, offset_elems, dims):
        """Manual AP: dims are [step, count] FREE dims; partition from base."""
        b = base if isinstance(base, bass.AP) else base[...]
        return bass.AP(
            tensor=b.tensor,
            offset=b.offset + offset_elems,
            ap=[list(b.ap[0])] + [list(d) for d in dims],
        )

    with tile.TileContext(nc) as tc:
        with tc.tile_pool(name="const", bufs=1) as const_pool:
            ident = const_pool.tile([128, 128], f32)
            make_identity(nc, ident)

            # Transposed weights, SBUF-resident for the whole kernel.
            # wt[p, kt, f] = W[f, kt*128 + p]   (for wk: f is permuted g-major)
            wts = {}
            with tc.tile_pool(name="wprep", bufs=2) as wnat_pool, \
                 tc.tile_pool(name="wtps", bufs=4, space="PSUM") as wt_psum:
                for name in ("wq", "wk", "wv", "wfc"):
                    wt = const_pool.tile([128, KT, H], f32r, tag=f"wt_{name}")
                    wts[name] = wt
                    wnat = wnat_pool.tile([128, KT, H], f32, tag="wnat")
                    nc.sync.dma_start(
                        out=wnat,
                        in_=w_d[name].rearrange("(ft p) c -> p ft c", p=128),
                    )
                    for ft in range(KT):
                        for kt in range(KT):
                            ps = wt_psum.tile([128, 128], f32, tag="wt_ps")
                            nc.tensor.transpose(
                                ps[:, :], wnat[:, ft, kt * 128:(kt + 1) * 128],
                                ident[:, :],
                            )
                            if name == "wk":
                                # permute output features to g-major:
                                # f = 16*dl + g + 128*ft  ->  f' = 64*g + 8*ft + dl
                                src = ap(ps, 0, [[16, 8], [1, 16]])       # (dl, g)
                                dst = ap(wt, kt * H + 8 * ft,
                                         [[1, 8], [64, 16]])              # (dl, g)
                                nc.scalar.copy(out=dst, in_=src)
                            else:
                                nc.scalar.copy(
                                    out=wt[:, kt, ft * 128:(ft + 1) * 128],
                                    in_=ps[:, :],
                                )

            with tc.tile_pool(name="io", bufs=2) as io_pool, \
                 tc.tile_pool(name="acts", bufs=2) as act_pool, \
                 tc.tile_pool(name="prod", bufs=2) as prod_pool, \
                 tc.tile_pool(name="small", bufs=2) as small_pool, \
                 tc.tile_pool(name="tps", bufs=4, space="PSUM") as t_psum, \
                 tc.tile_pool(name="mmps", bufs=4, space="PSUM") as mm_psum:

                for it in range(ntiles):
                    r0 = it * 128

                    h_sb = io_pool.tile([128, H], f32, tag="h")
                    nc.sync.dma_start(out=h_sb, in_=h_d[r0:r0 + 128, :])

                    # hT[p, c, j] = h[r0 + j, c*128 + p]
                    hT = act_pool.tile([128, KT, 128], f32r, tag="hT")
                    for c in range(KT if "tpose" not in ablate else 0):
                        ps = t_psum.tile([128, 128], f32, tag="tp")
                        nc.tensor.transpose(
                            ps[:, :], h_sb[:, c * 128:(c + 1) * 128], ident[:, :]
                        )
                        nc.scalar.copy(out=hT[:, c, :], in_=ps[:, :])

                    # Projections -> bf16 activations.
                    # qb: (hh, d) row-major;  kb: (g, d) [via permuted wk];
                    # vb: (d, g) row-major.
                    projs = {}
                    for name, pname in (("wq", "qb"), ("wk", "kb"), ("wv", "vb")):
                        dst = act_pool.tile([128, H], bf16, tag=pname)
                        projs[pname] = dst
                        for half in range(2):
                            ps = mm_psum.tile([128, 512], f32, tag="mm")
                            for kt in range(KT if "mm" not in ablate else 0):
                                nc.tensor.matmul(
                                    ps[:, :],
                                    hT[:, kt, :],
                                    wts[name][:, kt, half * 512:(half + 1) * 512],
                                    start=(kt == 0),
                                    stop=(kt == KT - 1),
                                )
                            nc.scalar.copy(
                                out=dst[:, half * 512:(half + 1) * 512], in_=ps[:, :]
                            )
                    qb, kb, vb = projs["qb"], projs["kb"], projs["vb"]

                    # comp[hh, g] = sum_d qb[hh*64+d] * kb[g*64+d]
                    comp = small_pool.tile([128, NH, NH], f32, tag="comp")
                    for qq in range(4 if "attn" not in ablate else 0):  # 4 heads per pass
                        p1 = prod_pool.tile([128, 4, NH, HD], bf16, tag="prod")
                        in0 = ap(qb, qq * 4 * HD, [[HD, 4], [0, NH], [1, HD]])
                        in1 = ap(kb, 0, [[0, 4], [HD, NH], [1, HD]])
                        nc.vector.tensor_tensor(p1[...], in0, in1, MULT)
                        # bf16 add-tree (2x mode) halves the 1x reduce cost:
                        # d: 64 -> 32 -> 16, then a short fp32 reduce over 16
                        tr = prod_pool.tile([128, 4096], bf16, tag="prod")
                        nc.vector.tensor_tensor(
                            ap(tr, 0, [[32, 64], [1, 32]]),
                            ap(p1, 0, [[64, 64], [1, 32]]),
                            ap(p1, 32, [[64, 64], [1, 32]]), ADD)
                        nc.vector.tensor_tensor(
                            ap(tr, 2048, [[16, 64], [1, 16]]),
                            ap(tr, 0, [[32, 64], [1, 16]]),
                            ap(tr, 16, [[32, 64], [1, 16]]), ADD)
                        nc.vector.tensor_reduce(
                            comp[:, qq * 4:(qq + 1) * 4, :],
                            ap(tr, 2048, [[16, 64], [1, 16]]), AXX, ADD
                        )

                    # softmax over g (scale by 1/128 inside exp), per hh-quarter
                    # so each quarter's scores unblock as soon as its comp lands
                    e = small_pool.tile([128, NH, NH], f32, tag="e")
                    s = small_pool.tile([128, NH], f32, tag="s")
                    r = small_pool.tile([128, NH], f32, tag="r")
                    scores = small_pool.tile([128, NH, NH], bf16, tag="sc")
                    for qq in range(4):
                        sl = slice(qq * 4, (qq + 1) * 4)
                        nc.scalar.activation(
                            e[:, sl, :], comp[:, sl, :],
                            mybir.ActivationFunctionType.Exp, scale=1.0 / 128.0,
                        )
                        nc.vector.tensor_reduce(s[:, sl], e[:, sl, :], AXX, ADD)
                        nc.vector.reciprocal(r[:, sl], s[:, sl])
                        nc.vector.tensor_tensor(
                            scores[:, sl, :], e[:, sl, :],
                            ap(r, qq * 4, [[1, 4], [0, NH]]), MULT
                        )

                    # out[l, d] = sum_g scores[l, g] * vb[d*16+g]
                    # OUT flat index = 16*d + l
                    OUT = act_pool.tile([128, H], f32, tag="out")
                    for dq in range(4 if "attn" not in ablate else 0):  # 16 d per pass
                        p2 = prod_pool.tile([128, NH, NH, NH], bf16, tag="prod")
                        in0 = ap(scores, 0, [[0, NH], [NH, NH], [1, NH]])
                        in1 = ap(vb, dq * NH * NH, [[NH, NH], [0, NH], [1, NH]])
                        nc.vector.tensor_tensor(p2[...], in0, in1, MULT)
                        # g: 16 -> 8 -> 4, then fp32 reduce over 4
                        tr = prod_pool.tile([128, 4096], bf16, tag="prod")
                        nc.vector.tensor_tensor(
                            ap(tr, 0, [[8, 256], [1, 8]]),
                            ap(p2, 0, [[16, 256], [1, 8]]),
                            ap(p2, 8, [[16, 256], [1, 8]]), ADD)
                        nc.vector.tensor_tensor(
                            ap(tr, 2048, [[4, 256], [1, 4]]),
                            ap(tr, 0, [[8, 256], [1, 4]]),
                            ap(tr, 4, [[8, 256], [1, 4]]), ADD)
                        nc.vector.tensor_reduce(
                            ap(OUT, dq * 256, [[1, 256]]).rearrange(
                                "p (a b) -> p a b", a=NH
                            ),
                            ap(tr, 2048, [[4, 256], [1, 4]]), AXX, ADD,
                        )

                    # outT[p, c, j] = OUT[j, c*128 + p]
                    outT = act_pool.tile([128, KT, 128], f32r, tag="outT")
                    for c in range(KT if "tpose" not in ablate else 0):
                        ps = t_psum.tile([128, 128], f32, tag="tp")
                        nc.tensor.transpose(
                            ps[:, :], OUT[:, c * 128:(c + 1) * 128], ident[:, :]
                        )
                        nc.scalar.copy(out=outT[:, c, :], in_=ps[:, :])

                    final = io_pool.tile([128, H], f32, tag="final")
                    for half in range(2):
                        ps = mm_psum.tile([128, 512], f32, tag="mm")
                        for kt in range(KT if "mm" not in ablate else 0):
                            nc.tensor.matmul(
                                ps[:, :],
                                outT[:, kt, :],
                                wts["wfc"][:, kt, half * 512:(half + 1) * 512],
                                start=(kt == 0),
                                stop=(kt == KT - 1),
                            )
                        nc.scalar.copy(
                            out=final[:, half * 512:(half + 1) * 512], in_=ps[:, :]
                        )
                    nc.sync.dma_start(out=out_d[r0:r0 + 128, :], in_=final)

    nc.compile()
    _BUILD_CACHE[key] = nc
    return nc


def kernel(h, Wq, Wk, Wv, Wfc):
    from concourse import bass_utils

    h = np.ascontiguousarray(np.asarray(h, dtype=np.float32))
    ws = {
        "wq": np.ascontiguousarray(np.asarray(Wq, dtype=np.float32)),
        "wk": np.ascontiguousarray(np.asarray(Wk, dtype=np.float32)),
        "wv": np.ascontiguousarray(np.asarray(Wv, dtype=np.float32)),
        "wfc": np.ascontiguousarray(np.asarray(Wfc, dtype=np.float32)),
    }
    nc = _build(NPC)
    in_maps = [
        {"h": h[i * NPC:(i + 1) * NPC], **ws} for i in range(NCORES)
    ]
    res = bass_utils.run_bass_kernel_spmd(nc, in_maps, core_ids=list(range(NCORES)))
    return np.concatenate(
        [res.results[i]["out"] for i in range(NCORES)], axis=0
    ).astype(np.float32)



# revision 8
# speedup vs baseline: 1.0283x; 1.0283x over previous
"""Trainium2 Bass kernel for per-node multi-head attention (v3).

Computation (per node n, fully independent across nodes):
    Q = h @ Wq.T  viewed (nh, hd)        [row-major reshape]
    K = h @ Wk.T  viewed (hd, nh)
    V = h @ Wv.T  viewed (hd, nh)
    comp[hh, g] = sum_d Q[hh, d] K[d, g] / 128
    scores = softmax(comp, axis=-1)
    out[l, d]  = sum_g scores[l, g] V[d, g]
    final = flat(out.T) @ Wfc.T

Sharding: data-parallel over N across 8 NeuronCores; no collectives.

v3 layout strategy (vs v1):
  - h is transposed AND pre-cast on the host: the kernel receives
    ht16 = h.T (bf16) and ht8 = h.T (fp8 e4m3).  No on-device h
    transposes or dtype casts.
  - Weights arrive pre-transposed (and Wk row-permuted g-major) so the
    projections are plain stationary=hT matmuls and there is no weight
    prep phase.
  - Q/K projections run as fp8 DoubleRow matmuls (2x PE throughput).
    Their quantization error is washed out by the near-uniform softmax
    (comp/128 has sigma ~0.026, so scores ~ 1/16 * (1 + eps)).
    V and the final FC stay bf16.
  - The attention einsums stay on VectorE (bf16 products at the 2x_1p
    rate) with a tunable number of passes offloaded to the Pool
    (gpsimd) engine, which is otherwise idle.
  - Output is written bf16 and upcast on the host.
"""

import numpy as np

N_FULL = 65536
H = 1024
NCORES = 8
NPC = N_FULL // NCORES  # rows per core
NH = 16                 # heads
HD = 64                 # head dim
KT = H // 128           # c chunks (8)
SLAB = 2                # tiles per h-input DMA slab

_BUILD_CACHE = {}


def _build(n_rows, cfg=None):
    key = (n_rows, tuple(sorted((cfg or {}).items())))
    if key in _BUILD_CACHE:
        return _BUILD_CACHE[key]
    cfg = cfg or {}
    # which einsum passes run fully on Pool (gpsimd): list of pass ids 0..7
    # (0-3 = einsum1 quarters, 4-7 = einsum2 quarters)
    # Pool may only take einsum2 work (pass ids 4..7): einsum1 on Pool would
    # make the softmax (and the whole DVE stream behind it) wait on the slow
    # Pool engine.
    pool_passes = cfg.get("pool_passes", (5,))
    # which passes get their add-tree run on Pool (DVE does products/reduce)
    pool_trees = cfg.get("pool_trees", ())
    # e2 passes whose FINAL add runs on Pool (the OUT consumer, the tail, runs
    # two tiles later, so Pool lag is harmless there)
    pool_finals = cfg.get("pool_finals", (4, 6, 7))
    # e2 passes whose product (mult) runs on Pool while DVE runs the tree
    pool_mults = cfg.get("pool_mults", ())

    import concourse.bass as bass
    import concourse.mybir as mybir
    import concourse.tile as tile
    from concourse import bacc
    from concourse.masks import make_identity

    f32 = mybir.dt.float32
    bf16 = mybir.dt.bfloat16
    f8 = mybir.dt.float8e4
    MULT = mybir.AluOpType.mult
    ADD = mybir.AluOpType.add
    AXX = mybir.AxisListType.X
    DR = mybir.MatmulPerfMode.DoubleRow

    nc = bacc.Bacc("TRN2", target_bir_lowering=False, debug=False)

    ht16_d = nc.dram_tensor("ht16", [H, n_rows], bf16, kind="ExternalInput").ap()
    ht8_d = nc.dram_tensor("ht8", [H, n_rows], f8, kind="ExternalInput").ap()
    wq8_d = nc.dram_tensor("wq8", [H, H], f8, kind="ExternalInput").ap()
    wk8_d = nc.dram_tensor("wk8", [H, H], f8, kind="ExternalInput").ap()
    wv16_d = nc.dram_tensor("wv16", [H, H], bf16, kind="ExternalInput").ap()
    wf16_d = nc.dram_tensor("wf16", [H, H], bf16, kind="ExternalInput").ap()
    out_d = nc.dram_tensor("out", [n_rows, H], bf16, kind="ExternalOutput").ap()

    ntiles = n_rows // 128
    nslabs = ntiles // SLAB

    def ap(base, offset_elems, dims):
        b = base if isinstance(base, bass.AP) else base[...]
        return bass.AP(
            tensor=b.tensor,
            offset=b.offset + offset_elems,
            ap=[list(b.ap[0])] + [list(d) for d in dims],
        )

    with tile.TileContext(nc) as tc:
        with tc.tile_pool(name="const", bufs=1) as const_pool:
            ident = const_pool.tile([128, 128], bf16)
            make_identity(nc, ident)

            # Persistent transposed weights (host pre-arranged).
            wq8 = const_pool.tile([128, KT, H], f8, tag="wq8")
            wk8 = const_pool.tile([128, KT, H], f8, tag="wk8")
            wv16 = const_pool.tile([128, KT, H], bf16, tag="wv16")
            wf16 = const_pool.tile([128, KT, H], bf16, tag="wf16")
            for dst, src in ((wq8, wq8_d), (wk8, wk8_d), (wv16, wv16_d),
                             (wf16, wf16_d)):
                nc.sync.dma_start(
                    out=dst, in_=src.rearrange("(ck p) f -> p ck f", p=128)
                )

            with tc.tile_pool(name="hin", bufs=2) as hin_pool, \
                 tc.tile_pool(name="acts", bufs=4) as act_pool, \
                 tc.tile_pool(name="prod", bufs=3) as prod_pool, \
                 tc.tile_pool(name="pprod", bufs=6) as pprod_pool, \
                 tc.tile_pool(name="small", bufs=4) as small_pool, \
                 tc.tile_pool(name="oarea", bufs=4) as o_pool, \
                 tc.tile_pool(name="otail", bufs=2) as otail_pool, \
                 tc.tile_pool(name="tps", bufs=2, space="PSUM") as t_psum, \
                 tc.tile_pool(name="mmps", bufs=6, space="PSUM") as mm_psum:

                def emit_tail(OUT, r0):
                    # transpose OUT -> outT, then the final FC + store
                    outT = otail_pool.tile([128, KT, 128], bf16, tag="outT")
                    for cq in range(2):
                        ps = t_psum.tile([128, 4, 128], bf16, tag="tp")
                        for ci in range(4):
                            c = cq * 4 + ci
                            nc.tensor.transpose(
                                ps[:, ci, :],
                                OUT[:, c * 128:(c + 1) * 128],
                                ident[:, :],
                            )
                        nc.scalar.copy(
                            out=outT[:, 4 * cq:4 * cq + 4, :], in_=ps[...])

                    fin = otail_pool.tile([128, H], bf16, tag="fin")
                    for half in range(2):
                        ps = mm_psum.tile([128, 512], f32, tag="mm")
                        for c in range(KT):
                            nc.tensor.matmul(
                                ps[:, :],
                                outT[:, c, :],
                                wf16[:, c, half * 512:(half + 1) * 512],
                                start=(c == 0),
                                stop=(c == KT - 1),
                            )
                        nc.scalar.copy(
                            out=fin[:, half * 512:(half + 1) * 512],
                            in_=ps[:, :],
                        )
                    nc.sync.dma_start(out=out_d[r0:r0 + 128, :], in_=fin)

                def emit_stage_a(hT16, hT8, st, r0):
                        ns = slice(st * 128, (st + 1) * 128)

                        # ---- projections ----
                        # Q, K: fp8 DoubleRow (contraction pairs of c-chunks)
                        projs = {}
                        for name, wt, pname in (("q", wq8, "qb"), ("k", wk8, "kb")):
                            dst = act_pool.tile([128, H], bf16, tag=pname)
                            projs[pname] = dst
                            for half in range(2):
                                ps = mm_psum.tile([128, 512], f32, tag="mm")
                                for dcp in range(4):
                                    nc.tensor.matmul(
                                        ps[:, :],
                                        hT8[:, 2 * dcp:2 * dcp + 2, ns],
                                        wt[:, 2 * dcp:2 * dcp + 2,
                                           half * 512:(half + 1) * 512],
                                        start=(dcp == 0),
                                        stop=(dcp == 3),
                                        perf_mode=DR,
                                    )
                                nc.scalar.copy(
                                    out=dst[:, half * 512:(half + 1) * 512],
                                    in_=ps[:, :],
                                )
                        qb, kb = projs["qb"], projs["kb"]

                        # ---- einsum1: comp[hh,g] = sum_d qb[hh*64+d]*kb[g*64+d]
                        # (kb is g-major via host permute of Wk rows)
                        # 2 passes of 8 heads each; all on DVE (Pool in the e1
                        # phase would stall the softmax behind it).
                        comp = small_pool.tile([128, NH, NH], f32, tag="comp")
                        for qq in range(2):
                            p1 = prod_pool.tile([128, 8, NH, HD], bf16, tag="prod")
                            in0 = ap(qb, qq * 8 * HD, [[HD, 8], [0, NH], [1, HD]])
                            in1 = ap(kb, 0, [[0, 8], [HD, NH], [1, HD]])
                            nc.vector.tensor_tensor(p1[...], in0, in1, MULT)
                            tr = prod_pool.tile([128, 8192], bf16, tag="prod")
                            # d: 64 -> 32 -> 16 -> 8 -> 4 -> 2 -> 1 (TT adds)
                            nc.vector.tensor_tensor(
                                ap(tr, 0, [[32, 128], [1, 32]]),
                                ap(p1, 0, [[64, 128], [1, 32]]),
                                ap(p1, 32, [[64, 128], [1, 32]]), ADD)
                            nc.vector.tensor_tensor(
                                ap(tr, 4096, [[16, 128], [1, 16]]),
                                ap(tr, 0, [[32, 128], [1, 16]]),
                                ap(tr, 16, [[32, 128], [1, 16]]), ADD)
                            nc.vector.tensor_tensor(
                                ap(tr, 6144, [[8, 128], [1, 8]]),
                                ap(tr, 4096, [[16, 128], [1, 8]]),
                                ap(tr, 4096 + 8, [[16, 128], [1, 8]]), ADD)
                            nc.vector.tensor_tensor(
                                ap(tr, 7168, [[4, 128], [1, 4]]),
                                ap(tr, 6144, [[8, 128], [1, 4]]),
                                ap(tr, 6144 + 4, [[8, 128], [1, 4]]), ADD)
                            nc.vector.tensor_tensor(
                                ap(tr, 7680, [[2, 128], [1, 2]]),
                                ap(tr, 7168, [[4, 128], [1, 2]]),
                                ap(tr, 7168 + 2, [[4, 128], [1, 2]]), ADD)
                            nc.vector.tensor_tensor(
                                comp[:, qq * 8:(qq + 1) * 8, :],
                                ap(tr, 7680, [[2, 128]]).rearrange(
                                    "p (a b) -> p a b", a=8),
                                ap(tr, 7681, [[2, 128]]).rearrange(
                                    "p (a b) -> p a b", a=8), ADD)

                        # exp runs on Act as part of stage A so e(t) is ready
                        # before the (one-tile-late) stage B needs it
                        e = small_pool.tile([128, NH, NH], bf16, tag="e")
                        nc.scalar.activation(
                            e[...], comp[...],
                            mybir.ActivationFunctionType.Exp,
                            scale=1.0 / 128.0,
                        )

                        # V projection emitted after exp so the Act stream
                        # reaches exp without queueing behind the vb copies
                        # (and behind PE's V matmuls). vb is consumed only by
                        # the one-tile-late stage B.
                        vb = act_pool.tile([128, H], bf16, tag="vb")
                        for half in range(2):
                            ps = mm_psum.tile([128, 512], f32, tag="mm")
                            for ck in range(KT):
                                nc.tensor.matmul(
                                    ps[:, :],
                                    hT16[:, ck, ns],
                                    wv16[:, ck, half * 512:(half + 1) * 512],
                                    start=(ck == 0),
                                    stop=(ck == KT - 1),
                                )
                            nc.scalar.copy(
                                out=vb[:, half * 512:(half + 1) * 512],
                                in_=ps[:, :],
                            )
                        return (e, vb, r0)

                def emit_stage_b(e, vb, r0):
                        # ---- softmax tail over g ----
                        s = small_pool.tile([128, NH], f32, tag="s")
                        r = small_pool.tile([128, NH], bf16, tag="r")
                        scores = small_pool.tile([128, NH, NH], bf16, tag="sc")
                        nc.vector.tensor_reduce(s[...], e[...], AXX, ADD)
                        with nc.allow_low_precision(
                                reason="r=1/s in bf16; 0.4% common-mode "
                                       "on scores is within budget"):
                            nc.vector.reciprocal(r[...], s[...])
                        nc.vector.tensor_tensor(
                            scores[...], e[...],
                            ap(r, 0, [[1, NH], [0, NH]]), MULT
                        )

                        # ---- einsum2: OUT[16d+l] = sum_g scores[l,g]*vb[16d+g]
                        OUT = o_pool.tile([128, H], bf16, tag="out")
                        for dq in range(4):
                            pid = 4 + dq
                            eng = (nc.gpsimd
                                   if pid in pool_passes or pid in pool_mults
                                   else nc.vector)
                            tree = (nc.gpsimd if (pid in pool_passes or
                                                  pid in pool_trees)
                                    else nc.vector)
                            pp = (pid in pool_passes or pid in pool_trees or
                                  pid in pool_mults)
                            pool_q = pprod_pool if pp else prod_pool
                            tr_pool = (pprod_pool if (pp or pid in pool_finals)
                                       else prod_pool)
                            p2 = pool_q.tile([128, NH, NH, NH], bf16, tag="prod")
                            in0 = ap(scores, 0, [[0, NH], [NH, NH], [1, NH]])
                            in1 = ap(vb, dq * NH * NH, [[NH, NH], [0, NH], [1, NH]])
                            eng.tensor_tensor(p2[...], in0, in1, MULT)
                            tr = tr_pool.tile([128, 4096], bf16, tag="prod")
                            # g: 16 -> 8 -> 4 -> 2 -> 1 (TT adds)
                            tree.tensor_tensor(
                                ap(tr, 0, [[8, 256], [1, 8]]),
                                ap(p2, 0, [[16, 256], [1, 8]]),
                                ap(p2, 8, [[16, 256], [1, 8]]), ADD)
                            tail_eng = (nc.gpsimd if pid in pool_finals
                                        else tree)
                            tail_eng.tensor_tensor(
                                ap(tr, 2048, [[4, 256], [1, 4]]),
                                ap(tr, 0, [[8, 256], [1, 4]]),
                                ap(tr, 4, [[8, 256], [1, 4]]), ADD)
                            tail_eng.tensor_tensor(
                                ap(tr, 3072, [[2, 256], [1, 2]]),
                                ap(tr, 2048, [[4, 256], [1, 2]]),
                                ap(tr, 2048 + 2, [[4, 256], [1, 2]]), ADD)
                            feng = (nc.gpsimd if pid in pool_finals else eng)
                            feng.tensor_tensor(
                                ap(OUT, dq * 256, [[1, 256]]),
                                ap(tr, 3072, [[2, 256]]),
                                ap(tr, 3073, [[2, 256]]), ADD)
                        return (OUT, r0)

                # Software pipeline: stage B (softmax tail + einsum2) runs one
                # tile behind stage A (proj + einsum1 + exp); the tail
                # (transpose + FC + store) two further tiles behind, so neither
                # the Act exp round-trip nor Pool's einsum2 lag ever stalls the
                # DVE / PE streams.
                def issue_slab_dma(sl):
                    hT16 = hin_pool.tile([128, KT, 128 * SLAB], bf16, tag="h16")
                    hT8 = hin_pool.tile([128, KT, 128 * SLAB], f8, tag="h8")
                    c0 = sl * 128 * SLAB
                    nc.sync.dma_start(
                        out=hT16,
                        in_=ht16_d[:, c0:c0 + 128 * SLAB].rearrange(
                            "(ck p) n -> p ck n", p=128),
                    )
                    nc.sync.dma_start(
                        out=hT8,
                        in_=ht8_d[:, c0:c0 + 128 * SLAB].rearrange(
                            "(ck p) n -> p ck n", p=128),
                    )
                    return hT16, hT8

                stA = stA2 = None
                tail = tail2 = None
                nxt = issue_slab_dma(0)
                for sl in range(nslabs):
                    hT16, hT8 = nxt
                    if sl + 1 < nslabs:
                        nxt = issue_slab_dma(sl + 1)
                    for st in range(SLAB):
                        it = sl * SLAB + st
                        stA_new = emit_stage_a(hT16, hT8, st, it * 128)
                        if stA2 is not None:
                            new_tail = emit_stage_b(*stA2)
                            if tail2 is not None:
                                emit_tail(*tail2)
                            tail2 = tail
                            tail = new_tail
                        stA2 = stA
                        stA = stA_new
                # drain
                for s in (stA2, stA):
                    if s is not None:
                        new_tail = emit_stage_b(*s)
                        if tail2 is not None:
                            emit_tail(*tail2)
                        tail2 = tail
                        tail = new_tail
                for t in (tail2, tail):
                    if t is not None:
                        emit_tail(*t)

    nc.compile()
    _BUILD_CACHE[key] = nc
    return nc


def _prep_inputs(h, Wq, Wk, Wv, Wfc):
    """Host-side layout prep. Returns per-core input dicts (shared weights)."""
    import concourse.mybir as mybir

    npf8 = mybir.dt.np(mybir.dt.float8e4)
    npbf = mybir.dt.np(mybir.dt.bfloat16)

    h = np.ascontiguousarray(np.asarray(h, dtype=np.float32))
    Wq = np.asarray(Wq, dtype=np.float32)
    Wk = np.asarray(Wk, dtype=np.float32)
    Wv = np.asarray(Wv, dtype=np.float32)
    Wfc = np.asarray(Wfc, dtype=np.float32)

    # Wk rows permuted g-major: kb[n, 64 g + d] = K[n, d, g] = kproj[n, 16 d + g]
    fprime = np.arange(H)
    perm_k = 16 * (fprime % 64) + (fprime // 64)   # row for feature f' = 64g+d
    wq8 = np.ascontiguousarray(Wq.T).astype(npf8)            # [c, f]
    wk8 = np.ascontiguousarray(Wk[perm_k].T).astype(npf8)    # [c, f'=64g+d]
    wv16 = np.ascontiguousarray(Wv.T).astype(npbf)           # [c, f=16d+g]
    wf16 = np.ascontiguousarray(Wfc.T).astype(npbf)          # [x=16d+l, f]

    ws = {"wq8": wq8, "wk8": wk8, "wv16": wv16, "wf16": wf16}
    in_maps = []
    for i in range(NCORES):
        hts = np.ascontiguousarray(h[i * NPC:(i + 1) * NPC].T)   # [H, NPC]
        in_maps.append({
            "ht16": hts.astype(npbf),
            "ht8": hts.astype(npf8),
            **ws,
        })
    return in_maps


def kernel(h, Wq, Wk, Wv, Wfc):
    from concourse import bass_utils

    nc = _build(NPC)
    in_maps = _prep_inputs(h, Wq, Wk, Wv, Wfc)
    res = bass_utils.run_bass_kernel_spmd(nc, in_maps, core_ids=list(range(NCORES)))
    return np.concatenate(
        [res.results[i]["out"].astype(np.float32) for i in range(NCORES)], axis=0
    )


# revision 12
# speedup vs baseline: 1.0472x; 1.0183x over previous
"""Trainium2 Bass kernel for per-node multi-head attention (v3).

Computation (per node n, fully independent across nodes):
    Q = h @ Wq.T  viewed (nh, hd)        [row-major reshape]
    K = h @ Wk.T  viewed (hd, nh)
    V = h @ Wv.T  viewed (hd, nh)
    comp[hh, g] = sum_d Q[hh, d] K[d, g] / 128
    scores = softmax(comp, axis=-1)
    out[l, d]  = sum_g scores[l, g] V[d, g]
    final = flat(out.T) @ Wfc.T

Sharding: data-parallel over N across 8 NeuronCores; no collectives.

v3 layout strategy (vs v1):
  - h is transposed AND pre-cast on the host: the kernel receives
    ht16 = h.T (bf16) and ht8 = h.T (fp8 e4m3).  No on-device h
    transposes or dtype casts.
  - Weights arrive pre-transposed (and Wk row-permuted g-major) so the
    projections are plain stationary=hT matmuls and there is no weight
    prep phase.
  - Q/K projections run as fp8 DoubleRow matmuls (2x PE throughput).
    Their quantization error is washed out by the near-uniform softmax
    (comp/128 has sigma ~0.026, so scores ~ 1/16 * (1 + eps)).
    V and the final FC stay bf16.
  - The attention einsums stay on VectorE (bf16 products at the 2x_1p
    rate) with a tunable number of passes offloaded to the Pool
    (gpsimd) engine, which is otherwise idle.
  - Output is written bf16 and upcast on the host.
"""

import numpy as np

N_FULL = 65536
H = 1024
NCORES = 8
NPC = N_FULL // NCORES  # rows per core
NH = 16                 # heads
HD = 64                 # head dim
KT = H // 128           # c chunks (8)
SLAB = 2                # tiles per h-input DMA slab

_BUILD_CACHE = {}


def _build(n_rows, cfg=None):
    key = (n_rows, tuple(sorted((cfg or {}).items())))
    if key in _BUILD_CACHE:
        return _BUILD_CACHE[key]
    cfg = cfg or {}
    # which einsum passes run fully on Pool (gpsimd): list of pass ids 0..7
    # (0-3 = einsum1 quarters, 4-7 = einsum2 quarters)
    # Pool may only take einsum2 work (pass ids 4..7): einsum1 on Pool would
    # make the softmax (and the whole DVE stream behind it) wait on the slow
    # Pool engine.
    pool_passes = cfg.get("pool_passes", (5,))
    # which passes get their add-tree run on Pool (DVE does products/reduce)
    pool_trees = cfg.get("pool_trees", ())
    # e2 passes whose FINAL add runs on Pool (the OUT consumer, the tail, runs
    # two tiles later, so Pool lag is harmless there)
    pool_finals = cfg.get("pool_finals", (4, 6, 7))
    # e2 passes whose product (mult) runs on Pool while DVE runs the tree
    pool_mults = cfg.get("pool_mults", ())

    import concourse.bass as bass
    import concourse.mybir as mybir
    import concourse.tile as tile
    from concourse import bacc
    from concourse.masks import make_identity

    f32 = mybir.dt.float32
    bf16 = mybir.dt.bfloat16
    f8 = mybir.dt.float8e4
    MULT = mybir.AluOpType.mult
    ADD = mybir.AluOpType.add
    AXX = mybir.AxisListType.X
    DR = mybir.MatmulPerfMode.DoubleRow

    nc = bacc.Bacc("TRN2", target_bir_lowering=False, debug=False)

    ht16_d = nc.dram_tensor("ht16", [H, n_rows], bf16, kind="ExternalInput").ap()
    ht8_d = nc.dram_tensor("ht8", [H, n_rows], f8, kind="ExternalInput").ap()
    wq8_d = nc.dram_tensor("wq8", [H, H], f8, kind="ExternalInput").ap()
    wk8_d = nc.dram_tensor("wk8", [H, H], f8, kind="ExternalInput").ap()
    wv16_d = nc.dram_tensor("wv16", [H, H], bf16, kind="ExternalInput").ap()
    wf16_d = nc.dram_tensor("wf16", [H, H], bf16, kind="ExternalInput").ap()
    out_d = nc.dram_tensor("out", [n_rows, H], bf16, kind="ExternalOutput").ap()

    ntiles = n_rows // 128
    nslabs = ntiles // SLAB

    def ap(base, offset_elems, dims):
        b = base if isinstance(base, bass.AP) else base[...]
        return bass.AP(
            tensor=b.tensor,
            offset=b.offset + offset_elems,
            ap=[list(b.ap[0])] + [list(d) for d in dims],
        )

    with tile.TileContext(nc) as tc:
        with tc.tile_pool(name="const", bufs=1) as const_pool:
            ident = const_pool.tile([128, 128], bf16)
            make_identity(nc, ident)

            # Persistent transposed weights (host pre-arranged).
            wq8 = const_pool.tile([128, KT, H], f8, tag="wq8")
            wk8 = const_pool.tile([128, KT, H], f8, tag="wk8")
            wv16 = const_pool.tile([128, KT, H], bf16, tag="wv16")
            wf16 = const_pool.tile([128, KT, H], bf16, tag="wf16")
            for dst, src in ((wq8, wq8_d), (wk8, wk8_d), (wv16, wv16_d),
                             (wf16, wf16_d)):
                nc.sync.dma_start(
                    out=dst, in_=src.rearrange("(ck p) f -> p ck f", p=128)
                )

            with tc.tile_pool(name="hin", bufs=2) as hin_pool, \
                 tc.tile_pool(name="acts", bufs=2) as act_pool, \
                 tc.tile_pool(name="vbp", bufs=5) as vb_pool, \
                 tc.tile_pool(name="prod", bufs=3) as prod_pool, \
                 tc.tile_pool(name="pprod", bufs=6) as pprod_pool, \
                 tc.tile_pool(name="small", bufs=5) as small_pool, \
                 tc.tile_pool(name="oarea", bufs=4) as o_pool, \
                 tc.tile_pool(name="otail", bufs=2) as otail_pool, \
                 tc.tile_pool(name="tps", bufs=2, space="PSUM") as t_psum, \
                 tc.tile_pool(name="mmps", bufs=6, space="PSUM") as mm_psum:

                def emit_tail(OUT, r0):
                    # transpose OUT -> outT, then the final FC + store
                    outT = otail_pool.tile([128, KT, 128], bf16, tag="outT")
                    for cq in range(2):
                        ps = t_psum.tile([128, 4, 128], bf16, tag="tp")
                        for ci in range(4):
                            c = cq * 4 + ci
                            nc.tensor.transpose(
                                ps[:, ci, :],
                                OUT[:, c * 128:(c + 1) * 128],
                                ident[:, :],
                            )
                        nc.scalar.copy(
                            out=outT[:, 4 * cq:4 * cq + 4, :], in_=ps[...])

                    fin = otail_pool.tile([128, H], bf16, tag="fin")
                    for half in range(2):
                        ps = mm_psum.tile([128, 512], f32, tag="mm")
                        for c in range(KT):
                            nc.tensor.matmul(
                                ps[:, :],
                                outT[:, c, :],
                                wf16[:, c, half * 512:(half + 1) * 512],
                                start=(c == 0),
                                stop=(c == KT - 1),
                            )
                        nc.scalar.copy(
                            out=fin[:, half * 512:(half + 1) * 512],
                            in_=ps[:, :],
                        )
                    nc.sync.dma_start(out=out_d[r0:r0 + 128, :], in_=fin)

                def emit_stage_a(hT16, hT8, st, r0):
                        ns = slice(st * 128, (st + 1) * 128)

                        # ---- projections ----
                        # Q, K: fp8 DoubleRow (contraction pairs of c-chunks)
                        projs = {}
                        for name, wt, pname in (("q", wq8, "qb"), ("k", wk8, "kb")):
                            dst = act_pool.tile([128, H], bf16, tag=pname)
                            projs[pname] = dst
                            for half in range(2):
                                ps = mm_psum.tile([128, 512], f32, tag="mm")
                                for dcp in range(4):
                                    nc.tensor.matmul(
                                        ps[:, :],
                                        hT8[:, 2 * dcp:2 * dcp + 2, ns],
                                        wt[:, 2 * dcp:2 * dcp + 2,
                                           half * 512:(half + 1) * 512],
                                        start=(dcp == 0),
                                        stop=(dcp == 3),
                                        perf_mode=DR,
                                    )
                                nc.scalar.copy(
                                    out=dst[:, half * 512:(half + 1) * 512],
                                    in_=ps[:, :],
                                )
                        qb, kb = projs["qb"], projs["kb"]

                        # ---- einsum1: comp[hh,g] = sum_d qb[hh*64+d]*kb[g*64+d]
                        # (kb is g-major via host permute of Wk rows)
                        # 2 passes of 8 heads each; all on DVE (Pool in the e1
                        # phase would stall the softmax behind it).
                        comp = small_pool.tile([128, NH, NH], f32, tag="comp")
                        for qq in range(2):
                            p1 = prod_pool.tile([128, 8, NH, HD], bf16, tag="prod")
                            in0 = ap(qb, qq * 8 * HD, [[HD, 8], [0, NH], [1, HD]])
                            in1 = ap(kb, 0, [[0, 8], [HD, NH], [1, HD]])
                            nc.vector.tensor_tensor(p1[...], in0, in1, MULT)
                            tr = prod_pool.tile([128, 8192], bf16, tag="prod")
                            # d: 64 -> 32 -> 16 -> 8 -> 4 -> 2 -> 1 (TT adds)
                            nc.vector.tensor_tensor(
                                ap(tr, 0, [[32, 128], [1, 32]]),
                                ap(p1, 0, [[64, 128], [1, 32]]),
                                ap(p1, 32, [[64, 128], [1, 32]]), ADD)
                            nc.vector.tensor_tensor(
                                ap(tr, 4096, [[16, 128], [1, 16]]),
                                ap(tr, 0, [[32, 128], [1, 16]]),
                                ap(tr, 16, [[32, 128], [1, 16]]), ADD)
                            nc.vector.tensor_tensor(
                                ap(tr, 6144, [[8, 128], [1, 8]]),
                                ap(tr, 4096, [[16, 128], [1, 8]]),
                                ap(tr, 4096 + 8, [[16, 128], [1, 8]]), ADD)
                            nc.vector.tensor_tensor(
                                ap(tr, 7168, [[4, 128], [1, 4]]),
                                ap(tr, 6144, [[8, 128], [1, 4]]),
                                ap(tr, 6144 + 4, [[8, 128], [1, 4]]), ADD)
                            nc.vector.tensor_tensor(
                                ap(tr, 7680, [[2, 128], [1, 2]]),
                                ap(tr, 7168, [[4, 128], [1, 2]]),
                                ap(tr, 7168 + 2, [[4, 128], [1, 2]]), ADD)
                            nc.vector.tensor_tensor(
                                comp[:, qq * 8:(qq + 1) * 8, :],
                                ap(tr, 7680, [[2, 128]]).rearrange(
                                    "p (a b) -> p a b", a=8),
                                ap(tr, 7681, [[2, 128]]).rearrange(
                                    "p (a b) -> p a b", a=8), ADD)

                        # exp runs on Act as part of stage A so e(t) is ready
                        # before the (one-tile-late) stage B needs it
                        e = small_pool.tile([128, NH, NH], bf16, tag="e")
                        nc.scalar.activation(
                            e[...], comp[...],
                            mybir.ActivationFunctionType.Exp,
                            scale=1.0 / 128.0,
                        )

                        # V projection emitted after exp so the Act stream
                        # reaches exp without queueing behind the vb copies
                        # (and behind PE's V matmuls). vb is consumed only by
                        # the one-tile-late stage B.
                        vb = vb_pool.tile([128, H], bf16, tag="vb")
                        for half in range(2):
                            ps = mm_psum.tile([128, 512], f32, tag="mm")
                            for ck in range(KT):
                                nc.tensor.matmul(
                                    ps[:, :],
                                    hT16[:, ck, ns],
                                    wv16[:, ck, half * 512:(half + 1) * 512],
                                    start=(ck == 0),
                                    stop=(ck == KT - 1),
                                )
                            nc.scalar.copy(
                                out=vb[:, half * 512:(half + 1) * 512],
                                in_=ps[:, :],
                            )
                        return (e, vb, r0)

                def emit_stage_b(e, vb, r0):
                        # ---- softmax tail over g ----
                        s = small_pool.tile([128, NH], f32, tag="s")
                        r = small_pool.tile([128, NH], bf16, tag="r")
                        scores = small_pool.tile([128, NH, NH], bf16, tag="sc")
                        nc.vector.tensor_reduce(s[...], e[...], AXX, ADD)
                        with nc.allow_low_precision(
                                reason="r=1/s in bf16; 0.4% common-mode "
                                       "on scores is within budget"):
                            nc.vector.reciprocal(r[...], s[...])
                        nc.vector.tensor_tensor(
                            scores[...], e[...],
                            ap(r, 0, [[1, NH], [0, NH]]), MULT
                        )

                        # ---- einsum2: OUT[16d+l] = sum_g scores[l,g]*vb[16d+g]
                        OUT = o_pool.tile([128, H], bf16, tag="out")
                        for dq in range(4):
                            pid = 4 + dq
                            eng = (nc.gpsimd
                                   if pid in pool_passes or pid in pool_mults
                                   else nc.vector)
                            tree = (nc.gpsimd if (pid in pool_passes or
                                                  pid in pool_trees)
                                    else nc.vector)
                            pp = (pid in pool_passes or pid in pool_trees or
                                  pid in pool_mults)
                            pool_q = pprod_pool if pp else prod_pool
                            tr_pool = (pprod_pool if (pp or pid in pool_finals)
                                       else prod_pool)
                            p2 = pool_q.tile([128, NH, NH, NH], bf16, tag="prod")
                            in0 = ap(scores, 0, [[0, NH], [NH, NH], [1, NH]])
                            in1 = ap(vb, dq * NH * NH, [[NH, NH], [0, NH], [1, NH]])
                            eng.tensor_tensor(p2[...], in0, in1, MULT)
                            tr = tr_pool.tile([128, 4096], bf16, tag="prod")
                            # g: 16 -> 8 -> 4 -> 2 -> 1 (TT adds)
                            tree.tensor_tensor(
                                ap(tr, 0, [[8, 256], [1, 8]]),
                                ap(p2, 0, [[16, 256], [1, 8]]),
                                ap(p2, 8, [[16, 256], [1, 8]]), ADD)
                            tail_eng = (nc.gpsimd if pid in pool_finals
                                        else tree)
                            tail_eng.tensor_tensor(
                                ap(tr, 2048, [[4, 256], [1, 4]]),
                                ap(tr, 0, [[8, 256], [1, 4]]),
                                ap(tr, 4, [[8, 256], [1, 4]]), ADD)
                            tail_eng.tensor_tensor(
                                ap(tr, 3072, [[2, 256], [1, 2]]),
                                ap(tr, 2048, [[4, 256], [1, 2]]),
                                ap(tr, 2048 + 2, [[4, 256], [1, 2]]), ADD)
                            feng = (nc.gpsimd if pid in pool_finals else eng)
                            feng.tensor_tensor(
                                ap(OUT, dq * 256, [[1, 256]]),
                                ap(tr, 3072, [[2, 256]]),
                                ap(tr, 3073, [[2, 256]]), ADD)
                        return (OUT, r0)

                # Software pipeline: stage B (softmax tail + einsum2) runs one
                # tile behind stage A (proj + einsum1 + exp); the tail
                # (transpose + FC + store) two further tiles behind, so neither
                # the Act exp round-trip nor Pool's einsum2 lag ever stalls the
                # DVE / PE streams.
                def issue_slab_dma(sl):
                    hT16 = hin_pool.tile([128, KT, 128 * SLAB], bf16, tag="h16")
                    hT8 = hin_pool.tile([128, KT, 128 * SLAB], f8, tag="h8")
                    c0 = sl * 128 * SLAB
                    nc.sync.dma_start(
                        out=hT16,
                        in_=ht16_d[:, c0:c0 + 128 * SLAB].rearrange(
                            "(ck p) n -> p ck n", p=128),
                    )
                    nc.sync.dma_start(
                        out=hT8,
                        in_=ht8_d[:, c0:c0 + 128 * SLAB].rearrange(
                            "(ck p) n -> p ck n", p=128),
                    )
                    return hT16, hT8

                from collections import deque
                pendA = deque()
                tails = deque()
                B_LAG = 3
                T_LAG = 2
                nxt = issue_slab_dma(0)
                for sl in range(nslabs):
                    hT16, hT8 = nxt
                    if sl + 1 < nslabs:
                        nxt = issue_slab_dma(sl + 1)
                    for st in range(SLAB):
                        it = sl * SLAB + st
                        pendA.append(emit_stage_a(hT16, hT8, st, it * 128))
                        if len(pendA) > B_LAG:
                            tails.append(emit_stage_b(*pendA.popleft()))
                        if len(tails) > T_LAG:
                            emit_tail(*tails.popleft())
                # drain
                while pendA:
                    tails.append(emit_stage_b(*pendA.popleft()))
                while tails:
                    emit_tail(*tails.popleft())

    nc.compile()
    _BUILD_CACHE[key] = nc
    return nc


def _prep_inputs(h, Wq, Wk, Wv, Wfc):
    """Host-side layout prep. Returns per-core input dicts (shared weights)."""
    import concourse.mybir as mybir

    npf8 = mybir.dt.np(mybir.dt.float8e4)
    npbf = mybir.dt.np(mybir.dt.bfloat16)

    h = np.ascontiguousarray(np.asarray(h, dtype=np.float32))
    Wq = np.asarray(Wq, dtype=np.float32)
    Wk = np.asarray(Wk, dtype=np.float32)
    Wv = np.asarray(Wv, dtype=np.float32)
    Wfc = np.asarray(Wfc, dtype=np.float32)

    # Wk rows permuted g-major: kb[n, 64 g + d] = K[n, d, g] = kproj[n, 16 d + g]
    fprime = np.arange(H)
    perm_k = 16 * (fprime % 64) + (fprime // 64)   # row for feature f' = 64g+d
    wq8 = np.ascontiguousarray(Wq.T).astype(npf8)            # [c, f]
    wk8 = np.ascontiguousarray(Wk[perm_k].T).astype(npf8)    # [c, f'=64g+d]
    wv16 = np.ascontiguousarray(Wv.T).astype(npbf)           # [c, f=16d+g]
    wf16 = np.ascontiguousarray(Wfc.T).astype(npbf)          # [x=16d+l, f]

    ws = {"wq8": wq8, "wk8": wk8, "wv16": wv16, "wf16": wf16}
    in_maps = []
    for i in range(NCORES):
        hts = np.ascontiguousarray(h[i * NPC:(i + 1) * NPC].T)   # [H, NPC]
        in_maps.append({
            "ht16": hts.astype(npbf),
            "ht8": hts.astype(npf8),
            **ws,
        })
    return in_maps


def kernel(h, Wq, Wk, Wv, Wfc):
    from concourse import bass_utils

    nc = _build(NPC)
    in_maps = _prep_inputs(h, Wq, Wk, Wv, Wfc)
    res = bass_utils.run_bass_kernel_spmd(nc, in_maps, core_ids=list(range(NCORES)))
    return np.concatenate(
        [res.results[i]["out"].astype(np.float32) for i in range(NCORES)], axis=0
    )


# revision 13
# speedup vs baseline: 1.0582x; 1.0106x over previous
"""Trainium2 Bass kernel for per-node multi-head attention (v3).

Computation (per node n, fully independent across nodes):
    Q = h @ Wq.T  viewed (nh, hd)        [row-major reshape]
    K = h @ Wk.T  viewed (hd, nh)
    V = h @ Wv.T  viewed (hd, nh)
    comp[hh, g] = sum_d Q[hh, d] K[d, g] / 128
    scores = softmax(comp, axis=-1)
    out[l, d]  = sum_g scores[l, g] V[d, g]
    final = flat(out.T) @ Wfc.T

Sharding: data-parallel over N across 8 NeuronCores; no collectives.

v3 layout strategy (vs v1):
  - h is transposed AND pre-cast on the host: the kernel receives
    ht16 = h.T (bf16) and ht8 = h.T (fp8 e4m3).  No on-device h
    transposes or dtype casts.
  - Weights arrive pre-transposed (and Wk row-permuted g-major) so the
    projections are plain stationary=hT matmuls and there is no weight
    prep phase.
  - Q/K projections run as fp8 DoubleRow matmuls (2x PE throughput).
    Their quantization error is washed out by the near-uniform softmax
    (comp/128 has sigma ~0.026, so scores ~ 1/16 * (1 + eps)).
    V and the final FC stay bf16.
  - The attention einsums stay on VectorE (bf16 products at the 2x_1p
    rate) with a tunable number of passes offloaded to the Pool
    (gpsimd) engine, which is otherwise idle.
  - Output is written bf16 and upcast on the host.
"""

import numpy as np

N_FULL = 65536
H = 1024
NCORES = 8
NPC = N_FULL // NCORES  # rows per core
NH = 16                 # heads
HD = 64                 # head dim
KT = H // 128           # c chunks (8)
SLAB = 2                # tiles per h-input DMA slab

_BUILD_CACHE = {}


def _build(n_rows, cfg=None):
    key = (n_rows, tuple(sorted((cfg or {}).items())))
    if key in _BUILD_CACHE:
        return _BUILD_CACHE[key]
    cfg = cfg or {}
    # which einsum passes run fully on Pool (gpsimd): list of pass ids 0..7
    # (0-3 = einsum1 quarters, 4-7 = einsum2 quarters)
    # Pool may only take einsum2 work (pass ids 4..7): einsum1 on Pool would
    # make the softmax (and the whole DVE stream behind it) wait on the slow
    # Pool engine.
    pool_passes = cfg.get("pool_passes", (5,))
    # which passes get their add-tree run on Pool (DVE does products/reduce)
    pool_trees = cfg.get("pool_trees", ())
    # e2 passes whose FINAL add runs on Pool (the OUT consumer, the tail, runs
    # two tiles later, so Pool lag is harmless there)
    pool_finals = cfg.get("pool_finals", (4, 6, 7))
    # e2 passes whose product (mult) runs on Pool while DVE runs the tree
    pool_mults = cfg.get("pool_mults", ())

    import concourse.bass as bass
    import concourse.mybir as mybir
    import concourse.tile as tile
    from concourse import bacc
    from concourse.masks import make_identity

    f32 = mybir.dt.float32
    bf16 = mybir.dt.bfloat16
    f8 = mybir.dt.float8e4
    MULT = mybir.AluOpType.mult
    ADD = mybir.AluOpType.add
    AXX = mybir.AxisListType.X
    DR = mybir.MatmulPerfMode.DoubleRow

    nc = bacc.Bacc("TRN2", target_bir_lowering=False, debug=False)

    ht16_d = nc.dram_tensor("ht16", [H, n_rows], bf16, kind="ExternalInput").ap()
    ht8_d = nc.dram_tensor("ht8", [H, n_rows], f8, kind="ExternalInput").ap()
    wq8_d = nc.dram_tensor("wq8", [H, H], f8, kind="ExternalInput").ap()
    wk8_d = nc.dram_tensor("wk8", [H, H], f8, kind="ExternalInput").ap()
    wv16_d = nc.dram_tensor("wv16", [H, H], bf16, kind="ExternalInput").ap()
    wf16_d = nc.dram_tensor("wf16", [H, H], bf16, kind="ExternalInput").ap()
    out_d = nc.dram_tensor("out", [n_rows, H], bf16, kind="ExternalOutput").ap()

    ntiles = n_rows // 128
    nslabs = ntiles // SLAB

    def ap(base, offset_elems, dims):
        b = base if isinstance(base, bass.AP) else base[...]
        return bass.AP(
            tensor=b.tensor,
            offset=b.offset + offset_elems,
            ap=[list(b.ap[0])] + [list(d) for d in dims],
        )

    with tile.TileContext(nc) as tc:
        with tc.tile_pool(name="const", bufs=1) as const_pool:
            ident = const_pool.tile([128, 128], bf16)
            make_identity(nc, ident)

            # Persistent transposed weights (host pre-arranged).
            wq8 = const_pool.tile([128, KT, H], f8, tag="wq8")
            wk8 = const_pool.tile([128, KT, H], f8, tag="wk8")
            wv16 = const_pool.tile([128, KT, H], bf16, tag="wv16")
            wf16 = const_pool.tile([128, KT, H], bf16, tag="wf16")
            for dst, src in ((wq8, wq8_d), (wk8, wk8_d), (wv16, wv16_d),
                             (wf16, wf16_d)):
                nc.sync.dma_start(
                    out=dst, in_=src.rearrange("(ck p) f -> p ck f", p=128)
                )

            with tc.tile_pool(name="hin", bufs=2) as hin_pool, \
                 tc.tile_pool(name="acts", bufs=2) as act_pool, \
                 tc.tile_pool(name="vbp", bufs=5) as vb_pool, \
                 tc.tile_pool(name="prod", bufs=3) as prod_pool, \
                 tc.tile_pool(name="pprod", bufs=6) as pprod_pool, \
                 tc.tile_pool(name="small", bufs=5) as small_pool, \
                 tc.tile_pool(name="oarea", bufs=4) as o_pool, \
                 tc.tile_pool(name="otail", bufs=2) as otail_pool, \
                 tc.tile_pool(name="tps", bufs=2, space="PSUM") as t_psum, \
                 tc.tile_pool(name="mmps", bufs=6, space="PSUM") as mm_psum:

                def emit_tail(OUT, r0):
                    # transpose OUT -> outT, then the final FC + store
                    outT = otail_pool.tile([128, KT, 128], bf16, tag="outT")
                    for cq in range(2):
                        ps = t_psum.tile([128, 4, 128], bf16, tag="tp")
                        for ci in range(4):
                            c = cq * 4 + ci
                            nc.tensor.transpose(
                                ps[:, ci, :],
                                OUT[:, c * 128:(c + 1) * 128],
                                ident[:, :],
                            )
                        nc.scalar.copy(
                            out=outT[:, 4 * cq:4 * cq + 4, :], in_=ps[...])

                    fin = otail_pool.tile([128, H], bf16, tag="fin")
                    for half in range(2):
                        ps = mm_psum.tile([128, 512], f32, tag="mm")
                        for c in range(KT):
                            nc.tensor.matmul(
                                ps[:, :],
                                outT[:, c, :],
                                wf16[:, c, half * 512:(half + 1) * 512],
                                start=(c == 0),
                                stop=(c == KT - 1),
                            )
                        nc.scalar.copy(
                            out=fin[:, half * 512:(half + 1) * 512],
                            in_=ps[:, :],
                        )
                    nc.sync.dma_start(out=out_d[r0:r0 + 128, :], in_=fin)

                def emit_stage_a(hT16, hT8, st, r0):
                        ns = slice(st * 128, (st + 1) * 128)

                        # ---- projections ----
                        # Q, K: fp8 DoubleRow (contraction pairs of c-chunks)
                        projs = {}
                        for name, wt, pname in (("q", wq8, "qb"), ("k", wk8, "kb")):
                            dst = act_pool.tile([128, H], bf16, tag=pname)
                            projs[pname] = dst
                            for half in range(2):
                                ps = mm_psum.tile([128, 512], f32, tag="mm")
                                for dcp in range(4):
                                    nc.tensor.matmul(
                                        ps[:, :],
                                        hT8[:, 2 * dcp:2 * dcp + 2, ns],
                                        wt[:, 2 * dcp:2 * dcp + 2,
                                           half * 512:(half + 1) * 512],
                                        start=(dcp == 0),
                                        stop=(dcp == 3),
                                        perf_mode=DR,
                                    )
                                nc.scalar.copy(
                                    out=dst[:, half * 512:(half + 1) * 512],
                                    in_=ps[:, :],
                                )
                        qb, kb = projs["qb"], projs["kb"]

                        # ---- einsum1: comp[hh,g] = sum_d qb[hh*64+d]*kb[g*64+d]
                        # (kb is g-major via host permute of Wk rows)
                        # 2 passes of 8 heads each; all on DVE (Pool in the e1
                        # phase would stall the softmax behind it).
                        comp = small_pool.tile([128, NH, NH], f32, tag="comp")
                        for qq in range(2):
                            p1 = prod_pool.tile([128, 8, NH, HD], bf16, tag="prod")
                            in0 = ap(qb, qq * 8 * HD, [[HD, 8], [0, NH], [1, HD]])
                            in1 = ap(kb, 0, [[0, 8], [HD, NH], [1, HD]])
                            nc.vector.tensor_tensor(p1[...], in0, in1, MULT)
                            tr = prod_pool.tile([128, 8192], bf16, tag="prod")
                            # d: 64 -> 32 -> 16 -> 8 -> 4 -> 2 -> 1 (TT adds)
                            nc.vector.tensor_tensor(
                                ap(tr, 0, [[32, 128], [1, 32]]),
                                ap(p1, 0, [[64, 128], [1, 32]]),
                                ap(p1, 32, [[64, 128], [1, 32]]), ADD)
                            nc.vector.tensor_tensor(
                                ap(tr, 4096, [[16, 128], [1, 16]]),
                                ap(tr, 0, [[32, 128], [1, 16]]),
                                ap(tr, 16, [[32, 128], [1, 16]]), ADD)
                            nc.vector.tensor_tensor(
                                ap(tr, 6144, [[8, 128], [1, 8]]),
                                ap(tr, 4096, [[16, 128], [1, 8]]),
                                ap(tr, 4096 + 8, [[16, 128], [1, 8]]), ADD)
                            nc.vector.tensor_tensor(
                                ap(tr, 7168, [[4, 128], [1, 4]]),
                                ap(tr, 6144, [[8, 128], [1, 4]]),
                                ap(tr, 6144 + 4, [[8, 128], [1, 4]]), ADD)
                            nc.vector.tensor_tensor(
                                ap(tr, 7680, [[2, 128], [1, 2]]),
                                ap(tr, 7168, [[4, 128], [1, 2]]),
                                ap(tr, 7168 + 2, [[4, 128], [1, 2]]), ADD)
                            nc.vector.tensor_tensor(
                                comp[:, qq * 8:(qq + 1) * 8, :],
                                ap(tr, 7680, [[2, 128]]).rearrange(
                                    "p (a b) -> p a b", a=8),
                                ap(tr, 7681, [[2, 128]]).rearrange(
                                    "p (a b) -> p a b", a=8), ADD)

                        # exp runs on Act as part of stage A so e(t) is ready
                        # before the (three-tile-late) stage B needs it.
                        # Per-head calls let accum_out produce the softmax
                        # denominators s[l] for free (no DVE TensorReduce).
                        e = small_pool.tile([128, NH, NH], bf16, tag="e")
                        s = small_pool.tile([128, NH], f32, tag="s")
                        for l in range(NH):
                            nc.scalar.activation(
                                e[:, l, :], comp[:, l, :],
                                mybir.ActivationFunctionType.Exp,
                                scale=1.0 / 128.0,
                                accum_out=s[:, l:l + 1],
                            )

                        # V projection emitted after exp so the Act stream
                        # reaches exp without queueing behind the vb copies
                        # (and behind PE's V matmuls). vb is consumed only by
                        # the one-tile-late stage B.
                        vb = vb_pool.tile([128, H], bf16, tag="vb")
                        for half in range(2):
                            ps = mm_psum.tile([128, 512], f32, tag="mm")
                            for ck in range(KT):
                                nc.tensor.matmul(
                                    ps[:, :],
                                    hT16[:, ck, ns],
                                    wv16[:, ck, half * 512:(half + 1) * 512],
                                    start=(ck == 0),
                                    stop=(ck == KT - 1),
                                )
                            nc.scalar.copy(
                                out=vb[:, half * 512:(half + 1) * 512],
                                in_=ps[:, :],
                            )
                        return (e, s, vb, r0)

                def emit_stage_b(e, s, vb, r0):
                        # ---- softmax tail over g ----
                        r = small_pool.tile([128, NH], bf16, tag="r")
                        scores = small_pool.tile([128, NH, NH], bf16, tag="sc")
                        with nc.allow_low_precision(
                                reason="r=1/s in bf16; 0.4% common-mode "
                                       "on scores is within budget"):
                            nc.vector.reciprocal(r[...], s[...])
                        nc.vector.tensor_tensor(
                            scores[...], e[...],
                            ap(r, 0, [[1, NH], [0, NH]]), MULT
                        )

                        # ---- einsum2: OUT[16d+l] = sum_g scores[l,g]*vb[16d+g]
                        OUT = o_pool.tile([128, H], bf16, tag="out")
                        for dq in range(4):
                            pid = 4 + dq
                            eng = (nc.gpsimd
                                   if pid in pool_passes or pid in pool_mults
                                   else nc.vector)
                            tree = (nc.gpsimd if (pid in pool_passes or
                                                  pid in pool_trees)
                                    else nc.vector)
                            pp = (pid in pool_passes or pid in pool_trees or
                                  pid in pool_mults)
                            pool_q = pprod_pool if pp else prod_pool
                            tr_pool = (pprod_pool if (pp or pid in pool_finals)
                                       else prod_pool)
                            p2 = pool_q.tile([128, NH, NH, NH], bf16, tag="prod")
                            in0 = ap(scores, 0, [[0, NH], [NH, NH], [1, NH]])
                            in1 = ap(vb, dq * NH * NH, [[NH, NH], [0, NH], [1, NH]])
                            eng.tensor_tensor(p2[...], in0, in1, MULT)
                            tr = tr_pool.tile([128, 4096], bf16, tag="prod")
                            # g: 16 -> 8 -> 4 -> 2 -> 1 (TT adds)
                            tree.tensor_tensor(
                                ap(tr, 0, [[8, 256], [1, 8]]),
                                ap(p2, 0, [[16, 256], [1, 8]]),
                                ap(p2, 8, [[16, 256], [1, 8]]), ADD)
                            tail_eng = (nc.gpsimd if pid in pool_finals
                                        else tree)
                            tail_eng.tensor_tensor(
                                ap(tr, 2048, [[4, 256], [1, 4]]),
                                ap(tr, 0, [[8, 256], [1, 4]]),
                                ap(tr, 4, [[8, 256], [1, 4]]), ADD)
                            tail_eng.tensor_tensor(
                                ap(tr, 3072, [[2, 256], [1, 2]]),
                                ap(tr, 2048, [[4, 256], [1, 2]]),
                                ap(tr, 2048 + 2, [[4, 256], [1, 2]]), ADD)
                            feng = (nc.gpsimd if pid in pool_finals else eng)
                            feng.tensor_tensor(
                                ap(OUT, dq * 256, [[1, 256]]),
                                ap(tr, 3072, [[2, 256]]),
                                ap(tr, 3073, [[2, 256]]), ADD)
                        return (OUT, r0)

                # Software pipeline: stage B (softmax tail + einsum2) runs one
                # tile behind stage A (proj + einsum1 + exp); the tail
                # (transpose + FC + store) two further tiles behind, so neither
                # the Act exp round-trip nor Pool's einsum2 lag ever stalls the
                # DVE / PE streams.
                def issue_slab_dma(sl):
                    hT16 = hin_pool.tile([128, KT, 128 * SLAB], bf16, tag="h16")
                    hT8 = hin_pool.tile([128, KT, 128 * SLAB], f8, tag="h8")
                    c0 = sl * 128 * SLAB
                    nc.sync.dma_start(
                        out=hT16,
                        in_=ht16_d[:, c0:c0 + 128 * SLAB].rearrange(
                            "(ck p) n -> p ck n", p=128),
                    )
                    nc.sync.dma_start(
                        out=hT8,
                        in_=ht8_d[:, c0:c0 + 128 * SLAB].rearrange(
                            "(ck p) n -> p ck n", p=128),
                    )
                    return hT16, hT8

                from collections import deque
                pendA = deque()
                tails = deque()
                B_LAG = 3
                T_LAG = 2
                nxt = issue_slab_dma(0)
                for sl in range(nslabs):
                    hT16, hT8 = nxt
                    if sl + 1 < nslabs:
                        nxt = issue_slab_dma(sl + 1)
                    for st in range(SLAB):
                        it = sl * SLAB + st
                        pendA.append(emit_stage_a(hT16, hT8, st, it * 128))
                        if len(pendA) > B_LAG:
                            tails.append(emit_stage_b(*pendA.popleft()))
                        if len(tails) > T_LAG:
                            emit_tail(*tails.popleft())
                # drain
                while pendA:
                    tails.append(emit_stage_b(*pendA.popleft()))
                while tails:
                    emit_tail(*tails.popleft())

    nc.compile()
    _BUILD_CACHE[key] = nc
    return nc


def _prep_inputs(h, Wq, Wk, Wv, Wfc):
    """Host-side layout prep. Returns per-core input dicts (shared weights)."""
    import concourse.mybir as mybir

    npf8 = mybir.dt.np(mybir.dt.float8e4)
    npbf = mybir.dt.np(mybir.dt.bfloat16)

    h = np.ascontiguousarray(np.asarray(h, dtype=np.float32))
    Wq = np.asarray(Wq, dtype=np.float32)
    Wk = np.asarray(Wk, dtype=np.float32)
    Wv = np.asarray(Wv, dtype=np.float32)
    Wfc = np.asarray(Wfc, dtype=np.float32)

    # Wk rows permuted g-major: kb[n, 64 g + d] = K[n, d, g] = kproj[n, 16 d + g]
    fprime = np.arange(H)
    perm_k = 16 * (fprime % 64) + (fprime // 64)   # row for feature f' = 64g+d
    wq8 = np.ascontiguousarray(Wq.T).astype(npf8)            # [c, f]
    wk8 = np.ascontiguousarray(Wk[perm_k].T).astype(npf8)    # [c, f'=64g+d]
    wv16 = np.ascontiguousarray(Wv.T).astype(npbf)           # [c, f=16d+g]
    wf16 = np.ascontiguousarray(Wfc.T).astype(npbf)          # [x=16d+l, f]

    ws = {"wq8": wq8, "wk8": wk8, "wv16": wv16, "wf16": wf16}
    in_maps = []
    for i in range(NCORES):
        hts = np.ascontiguousarray(h[i * NPC:(i + 1) * NPC].T)   # [H, NPC]
        in_maps.append({
            "ht16": hts.astype(npbf),
            "ht8": hts.astype(npf8),
            **ws,
        })
    return in_maps


def kernel(h, Wq, Wk, Wv, Wfc):
    from concourse import bass_utils

    nc = _build(NPC)
    in_maps = _prep_inputs(h, Wq, Wk, Wv, Wfc)
    res = bass_utils.run_bass_kernel_spmd(nc, in_maps, core_ids=list(range(NCORES)))
    return np.concatenate(
        [res.results[i]["out"].astype(np.float32) for i in range(NCORES)], axis=0
    )
